# revision 2
# baseline (speedup 1.0000x reference)
"""Trainium2 Bass kernel v2 for 16-head causal self-attention (KaplanAttention).

Problem: x [2, 2048, 1024], torch-style weights W_q/W_k/W_v/W_o [1024, 1024].
  q/k/v = (x @ W.T) split into 16 heads of 64; causal softmax(q k^T / 8) @ v;
  concat heads; out = attn_out @ W_o.T.

Sharding (8 cores): core c handles batch b = c // 4 and head group g = c % 4
(heads 4g..4g+3). Each core computes its 4 heads' attention output and a
partial output projection against the matching 256 columns of W_o; the host
sums the 4 partials per batch (fp32) — the "all-reduce" of the row-sharded W_o.

v2 structure (flash-style streaming over query s-tiles, pair-interleaved):
  for t in 0..3 (512 queries each):
    project Q/K s-tile t and V j-tiles 4t..4t+3 (so projection PE work
    interleaves with attention of earlier tiles);
    for jc in 0..4t+3 (128 keys), for pair hp in {0,1}:
      scores^T psum [128 j, 2 heads, 512 s] (2 banks, bufs=2 so ACT(exp) of
      one pair overlaps PE(scores/AV) of the other); causal = ragged cols
      (s >= 128*jc) + ADDITIVE -30000 mask matmul (lhsT=maskb, rhs=I) on the
      diagonal 128-block, so exp gives exact zeros — no vector masking.
      exp(scale=1/8) ACT -> UT ring fp16; AV accumulates [65, 512] fp32 per
      head across jc (V has a ones column so row 64 = softmax denominator Z).
    normalize per head (reciprocal + gpsimd partition_broadcast + mul);
    final projection per s-128-chunk; fp16 partial out (host sums in fp32).
"""

import numpy as np

from concourse import bass_utils, mybir, tile
from concourse import bacc

S = 2048
D = 1024
HPC = 4        # heads per core
DK = 64
DC = HPC * DK  # 256 d-columns per core
NCORES = 8
EC = D // 128   # 8 e-chunks
NJT = S // 128  # 16 j-tiles
NST = S // 512  # 4 s-tiles

FP16 = mybir.dt.float16
FP32 = mybir.dt.float32

NEG = -30000.0  # additive causal mask (pre-scale); exp(0.125*(x+NEG)) == 0


def _build(reps=1):
    nc = bacc.Bacc("TRN2", target_bir_lowering=False, debug=False)

    xT_d = nc.dram_tensor("xT", [D, S], FP16, kind="ExternalInput")
    wq_d = nc.dram_tensor("wqT", [D, DC], FP16, kind="ExternalInput")
    wk_d = nc.dram_tensor("wkT", [D, DC], FP16, kind="ExternalInput")
    wv_d = nc.dram_tensor("wvT", [D, DC], FP16, kind="ExternalInput")
    wo_d = nc.dram_tensor("woT", [DC, D], FP16, kind="ExternalInput")
    mb_d = nc.dram_tensor("maskb", [128, 128], FP16, kind="ExternalInput")
    id_d = nc.dram_tensor("ident", [128, 128], FP16, kind="ExternalInput")
    out_d = nc.dram_tensor("out", [S, D], FP16, kind="ExternalOutput")

    with tile.TileContext(nc) as tc:
        with (
            tc.tile_pool(name="const", bufs=1) as const,
            tc.tile_pool(name="work", bufs=1) as work,
            tc.tile_pool(name="ut", bufs=4) as utp,
            tc.tile_pool(name="outs", bufs=2) as outsp,
            tc.tile_pool(name="norm", bufs=4) as normp,
            tc.tile_pool(name="psS", bufs=2, space="PSUM") as psS,
            tc.tile_pool(name="psA", bufs=2, space="PSUM") as psA,
            tc.tile_pool(name="psP", bufs=2, space="PSUM") as psP,
        ):
          for _rep in range(reps):
            # ---- input DMAs, ordered so t=0 work is ready ASAP ----
            xT = const.tile([128, EC, S], FP16)
            xT_src = xT_d.rearrange("(c p) s -> p c s", p=128)
            wq = const.tile([128, EC, DC], FP16)
            wk = const.tile([128, EC, DC], FP16)
            wv = const.tile([128, EC, DC], FP16)
            nc.scalar.dma_start(out=wq, in_=wq_d.rearrange("(c p) d -> p c d", p=128))
            nc.sync.dma_start(out=xT[:, :, 0:512], in_=xT_src[:, :, 0:512])
            for w_t, w_dr in ((wk, wk_d), (wv, wv_d)):
                nc.scalar.dma_start(out=w_t, in_=w_dr.rearrange("(c p) d -> p c d", p=128))
            for q in range(1, 4):
                nc.sync.dma_start(
                    out=xT[:, :, 512 * q : 512 * (q + 1)],
                    in_=xT_src[:, :, 512 * q : 512 * (q + 1)],
                )
            wo = const.tile([128, 2, D], FP16)
            nc.scalar.dma_start(out=wo, in_=wo_d.rearrange("(c p) d -> p c d", p=128))
            maskb = const.tile([128, 128], FP16)
            nc.scalar.dma_start(out=maskb, in_=mb_d[:, :])
            ident = const.tile([128, 128], FP16)
            nc.scalar.dma_start(out=ident, in_=id_d[:, :])

            ld = mybir.InstLoadActFuncSet(
                name=f"I-{nc.next_id()}", ins=[], outs=[], act_func_set_id=6
            )
            nc.scalar.add_instruction(ld)

            QT = work.tile([128, 2, S], FP16)  # head h of pair hp at partitions 64*(h%2)
            KT = work.tile([128, 2, S], FP16)
            V = work.tile([128, NJT, HPC, 65], FP16)  # col 64 = 1.0 (Z trick)
            nc.vector.memset(V[:, :, :, 64:65], 1.0)
            outTn = work.tile([128, 2, S], FP16)  # normalized out^T, pair-stacked

            def proj_steps(t):
                """Yield closures, each emitting one projection matmul (or a
                closing psum->SBUF copy) for s-tile t; consumed one per jc
                slot inside the attention loop so the static PE stream has
                filler during ACT(exp)-paced stretches."""
                s0 = 512 * t
                state = {}

                def qk_mm(w_t, hp, c, key):
                    def emit():
                        if c == 0:
                            state[key] = psP.tile(
                                [128, 512], FP32, tag="proj", name=f"pj_{key}"
                            )
                        nc.tensor.matmul(
                            state[key],
                            w_t[:, c, 128 * hp : 128 * (hp + 1)],
                            xT[:, c, s0 : s0 + 512],
                            start=(c == 0),
                            stop=(c == EC - 1),
                        )
                    return emit

                def qk_copy(dst, hp, key):
                    def emit():
                        nc.vector.tensor_copy(
                            out=dst[:, hp, s0 : s0 + 512], in_=state[key]
                        )
                    return emit

                def v_mm(jt, c, key):
                    def emit():
                        if c == 0:
                            state[key] = psP.tile(
                                [128, 512], FP32, tag="proj", name=f"pv_{key}"
                            )
                        nc.tensor.matmul(
                            state[key][:, 0:DC],
                            xT[:, c, 128 * jt : 128 * (jt + 1)],
                            wv[:, c, :],
                            start=(c == 0),
                            stop=(c == EC - 1),
                        )
                    return emit

                def v_copy(jt, key):
                    def emit():
                        nc.vector.tensor_copy(
                            out=V[:, jt, :, 0:64],
                            in_=state[key][:, 0:DC].rearrange(
                                "p (h d) -> p h d", h=HPC
                            ),
                        )
                    return emit

                def qk_chain(hp):
                    for w_i, (w_t, dst) in enumerate(((wq, QT), (wk, KT))):
                        key = f"{t}_{w_i}_{hp}"
                        for c in range(EC):
                            yield qk_mm(w_t, hp, c, key)
                        yield qk_copy(dst, hp, key)

                yield from qk_chain(0)
                for jt in range(4 * t, 4 * t + 4):
                    key = f"v{jt}"
                    for c in range(EC):
                        yield v_mm(jt, c, key)
                    yield v_copy(jt, key)
                yield from qk_chain(1)

            def run_steps(steps, k):
                for _ in range(k):
                    try:
                        next(steps)()
                    except StopIteration:
                        return False
                return True

            def final_steps(t):
                """Final projection of s-tile t, one closure per matmul/copy,
                plus the closing output DMA; streamed through the filler."""
                state = {}
                ob = outsp.tile([128, 4, D], FP16, tag="ob", name=f"ob_{t}")

                def fmm(st, mt, hp, key):
                    def emit():
                        if hp == 0:
                            state[key] = psP.tile(
                                [128, 512], FP32, tag="proj", name=f"psf_{key}"
                            )
                        nc.tensor.matmul(
                            state[key],
                            outTn[:, hp, 128 * st : 128 * (st + 1)],
                            wo[:, hp, 512 * mt : 512 * (mt + 1)],
                            start=(hp == 0),
                            stop=(hp == 1),
                        )
                    return emit

                def fcopy(ci, mt, key):
                    def emit():
                        nc.vector.tensor_copy(
                            out=ob[:, ci, 512 * mt : 512 * (mt + 1)],
                            in_=state[key],
                        )
                    return emit

                for ci in range(4):
                    st = 4 * t + ci
                    for mt in range(2):
                        key = f"{st}_{mt}"
                        yield fmm(st, mt, 0, key)
                        yield fmm(st, mt, 1, key)
                        yield fcopy(ci, mt, key)

                def do_dma():
                    nc.sync.dma_start(
                        out=out_d.rearrange("(k p) m -> p k m", p=128)[
                            :, 4 * t : 4 * t + 4, :
                        ],
                        in_=ob,
                    )
                yield do_dma

            import itertools

            steps = proj_steps(0)
            while run_steps(steps, 1):
                pass

            for t in range(NST):
                s0 = 512 * t
                gens = []
                if t >= 1:
                    gens.append(final_steps(t - 1))
                if t + 1 < NST:
                    gens.append(proj_steps(t + 1))
                steps = itertools.chain(*gens)
                # ---- attention: each head-pair sweeps all key chunks ----
                njc = 4 * t + 4
                def emit_scores(hp, jc):
                    i = jc - 4 * t
                    off = 128 * i if i >= 0 else 0
                    n = 512 - off
                    ps = psS.tile([128, 2, 512], FP32, tag="score",
                                  name=f"ps_{t}_{jc}_{hp}")
                    for hi in range(2):
                        ho = 64 * hi
                        nc.tensor.matmul(
                            ps[:, hi, off : off + n],
                            KT[ho : ho + 64, hp, 128 * jc : 128 * (jc + 1)],
                            QT[ho : ho + 64, hp, s0 + off : s0 + 512],
                            start=True,
                            stop=(i < 0),
                        )
                        if i >= 0:
                            nc.tensor.matmul(
                                ps[:, hi, off : off + 128],
                                maskb,
                                ident,
                                start=False,
                                stop=True,
                            )
                    return ps, off, n

                slots = [(hp, jc) for hp in range(2) for jc in range(njc)]
                psa_by_hp = {}
                pend = emit_scores(*slots[0])
                for k, (hp, jc) in enumerate(slots):
                    if jc == 0:
                        psa_by_hp[hp] = [
                            psA.tile([65, 512], FP32, tag="av",
                                     name=f"psa_{t}_{hp}_{hi}")
                            for hi in range(2)
                        ]
                    psa = psa_by_hp[hp]
                    ps, off, n = pend
                    ut = utp.tile([128, 2, 512], FP16, tag="ut",
                                  name=f"ut_{t}_{jc}_{hp}")
                    nc.scalar.activation(
                        out=ut[:, :, off : off + n],
                        in_=ps[:, :, off : off + n],
                        func=mybir.ActivationFunctionType.Exp,
                        scale=0.125,
                    )
                    # scores for the NEXT slot go into the PE stream BEFORE
                    # this slot's AV (which waits on exp) — keeps the exp
                    # chain continuous instead of serializing exp->AV->S->exp
                    if k + 1 < len(slots):
                        pend = emit_scores(*slots[k + 1])
                    for hi in range(2):
                        h = 2 * hp + hi
                        nc.tensor.matmul(
                            psa[hi][:, off : off + n],
                            V[:, jc, h, :],
                            ut[:, hi, off : off + n],
                            start=(jc == 0),
                            stop=(jc == njc - 1),
                        )
                    if k % 4 == 0:
                        run_steps(steps, 9)
                    if jc == njc - 1:
                        # normalize this pair: evacuate AV psum, then
                        # recip -> gpsimd broadcast -> mul off the PE path
                        for hi in range(2):
                            ho = 64 * hi
                            # 1/Z = exp(-ln Z): the DVE reciprocal is an
                            # 8-pass iterative divide (~7us for 512 cols on
                            # HW); two ACT LUT ops are ~0.7us each and both
                            # live in act table set 6 (preloaded once).
                            zl = normp.tile([1, 512], FP32, tag="zln",
                                            name=f"zl_{t}_{hp}_{hi}")
                            nc.scalar.activation(
                                out=zl,
                                in_=psa[hi][64:65, :],
                                func=mybir.ActivationFunctionType.Ln,
                            )
                            zr = normp.tile([1, 512], FP32, tag="zrow",
                                            name=f"zr_{t}_{hp}_{hi}")
                            nc.scalar.activation(
                                out=zr,
                                in_=zl,
                                func=mybir.ActivationFunctionType.Exp,
                                scale=-1.0,
                            )
                            zb = normp.tile([64, 512], FP32, tag="zb",
                                            name=f"zb_{t}_{hp}_{hi}")
                            nc.gpsimd.partition_broadcast(zb, zr)
                            nc.vector.tensor_mul(
                                outTn[ho : ho + 64, hp, s0 : s0 + 512],
                                psa[hi][0:64, :],
                                zb,
                            )
                while run_steps(steps, 4):
                    pass

            fin = final_steps(NST - 1)
            while run_steps(fin, 4):
                pass

    nc.compile()
    return nc


_NC = None


def _prep_in_maps(x, W_q, W_k, W_v, W_o):
    x = np.asarray(x, dtype=np.float32)
    W_q = np.asarray(W_q, dtype=np.float32)
    W_k = np.asarray(W_k, dtype=np.float32)
    W_v = np.asarray(W_v, dtype=np.float32)
    W_o = np.asarray(W_o, dtype=np.float32)
    # additive mask, pre-transposed for the PE: lhsT[s, j] = NEG where s < j
    maskb = (np.triu(np.ones((128, 128), dtype=np.float32), k=1) * NEG).astype(
        np.float16
    )
    ident = np.eye(128, dtype=np.float16)
    in_maps = []
    for c in range(NCORES):
        b, g = divmod(c, 4)
        cols = slice(DC * g, DC * (g + 1))
        in_maps.append(
            {
                "xT": np.ascontiguousarray(x[b].T).astype(np.float16),
                "wqT": np.ascontiguousarray(W_q[cols, :].T).astype(np.float16),
                "wkT": np.ascontiguousarray(W_k[cols, :].T).astype(np.float16),
                "wvT": np.ascontiguousarray(W_v[cols, :].T).astype(np.float16),
                "woT": np.ascontiguousarray(W_o[:, cols].T).astype(np.float16),
                "maskb": maskb,
                "ident": ident,
            }
        )
    return in_maps


def _run(x, W_q, W_k, W_v, W_o, **spmd_kwargs):
    global _NC
    if _NC is None:
        _NC = _build()
    in_maps = _prep_in_maps(x, W_q, W_k, W_v, W_o)
    res = bass_utils.run_bass_kernel_spmd(
        _NC, in_maps, core_ids=list(range(NCORES)), **spmd_kwargs
    )
    parts = [res.results[c]["out"].astype(np.float32) for c in range(NCORES)]
    out = np.empty((2, S, D), dtype=np.float32)
    for b in range(2):
        out[b] = parts[4 * b] + parts[4 * b + 1] + parts[4 * b + 2] + parts[4 * b + 3]
    return out, res


def kernel(x, W_q, W_k, W_v, W_o):
    out, _ = _run(x, W_q, W_k, W_v, W_o)
    return out
